# revision 94
# baseline (speedup 1.0000x reference)
"""Masked multi-head attention (B=8, N=1024, C=768, H=12) on 8 trn2 NeuronCores.

Sharding: pure data-parallel over batch - core i computes batch element i
end-to-end (qkv linear, masked softmax attention, output projection).
No collectives.

Device-side design (v2):
  qkT    [2C, N]   q/k head-dims on partitions, produced by matmul; bias
                   added during PSUM evacuation (DVE).
  S.T    [m, n] per (head, key-tile): keys on partitions. No max-subtraction
         (scores are O(1)); exp on ACT.
  em     exp * maskT on DVE (a share on gpsimd, which is SBUF-only).
  EV     FLIPPED: psE[q, d] = sum_m em[m, q] * v_ext[m, d] with em as the
         stationary operand -> out free size 65 per matmul (half the PE
         columns of the [d, q] orientation). Column 64 of v_ext is ones, so
         psE[:, 64] is the softmax denominator, per-partition -> reciprocal
         and normalize fuse into the evacuation (tensor_scalar_mul).
  attn   [q, f] normalized, bf16; PE-transposed per 128x128 block into
         attnT [f, q] for the projection.
  projT  out.T [fo, n] accumulated over fin; proj bias (with the host-folded
         v-bias term pb' = pb + proj_w @ vb) added during ACT evacuation.
Host pre-transposes/casts x, mask, qkv_w, proj_w; output comes back as
out.T and is transposed on host. All matmuls bf16 with fp32 PSUM.
"""

import numpy as np
import ml_dtypes

import concourse.bass as bass
import concourse.mybir as mybir
import concourse.tile as tile
from concourse import bacc
from concourse.bass_utils import run_bass_kernel_spmd

B, N, C, H = 8, 1024, 768, 12
D = C // H  # 64
SCALE = 0.125
NT = N // 128  # 8 n-tiles
CT = C // 128  # 6 c-tiles
BF16 = mybir.dt.bfloat16
F32 = mybir.dt.float32
NPBF16 = ml_dtypes.bfloat16

_CACHE: dict = {}


def _build_bass():
    nc = bacc.Bacc(None, target_bir_lowering=False, debug=False)

    xT_d = nc.dram_tensor("xT", [C, N], BF16, kind="ExternalInput")
    maskT_d = nc.dram_tensor("maskT", [N, N], BF16, kind="ExternalInput")
    wT_d = nc.dram_tensor("qkv_wT", [C, 3 * C], BF16, kind="ExternalInput")
    pwT_d = nc.dram_tensor("proj_wT", [C, C], BF16, kind="ExternalInput")
    qb_d = nc.dram_tensor("qb_col", [128, 12], F32, kind="ExternalInput")
    pb_d = nc.dram_tensor("pb_col", [128, CT], F32, kind="ExternalInput")
    id_d = nc.dram_tensor("ident", [128, 128], BF16, kind="ExternalInput")
    out_d = nc.dram_tensor("outT", [C, N], F32, kind="ExternalOutput")

    with tile.TileContext(nc) as tc:
        _emit(nc, tc, xT_d, maskT_d, wT_d, pwT_d, qb_d, pb_d, id_d, out_d)
    nc.compile()
    return nc


def _emit(nc, tc, xT_d, maskT_d, wT_d, pwT_d, qb_d, pb_d, id_d, out_d):
    Exp = mybir.ActivationFunctionType.Exp
    Ident = mybir.ActivationFunctionType.Identity

    with (
        tc.tile_pool(name="consts", bufs=1) as consts,
        tc.tile_pool(name="work", bufs=4) as work,
    ):
        # ---- persistent SBUF residents -------------------------------
        xT = consts.tile([128, CT, N], BF16, name="xT_sb")
        wT = consts.tile([128, CT, 3 * C], BF16, name="wT_sb")
        pwT = consts.tile([128, CT, C], BF16, name="pwT_sb")
        maskT = consts.tile([128, NT, N], BF16, name="maskT_sb")
        qb = consts.tile([128, 12], F32, name="qb_sb")
        pb = consts.tile([128, CT], F32, name="pb_sb")
        ident = consts.tile([128, 128], BF16, name="id_sb")
        qkT = consts.tile([128, 12, N], BF16, name="qkT_sb")
        v_ext = consts.tile([128, NT, H, D + 1], BF16, name="vext_sb")
        attn = consts.tile([128, NT, C], BF16, name="attn_sb")
        attnT = consts.tile([128, CT, N], BF16, name="attnT_sb")

        xT_r = xT_d.ap().rearrange("(t p) n -> p t n", p=128)
        wT_r = wT_d.ap().rearrange("(t p) n -> p t n", p=128)
        maskT_r = maskT_d.ap().rearrange("(t p) n -> p t n", p=128)
        pwT_r = pwT_d.ap().rearrange("(t p) n -> p t n", p=128)
        # tiny const loads on the ACT HWDGE ring, big loads on the SP ring
        nc.scalar.dma_start(out=qb, in_=qb_d.ap())
        nc.scalar.dma_start(out=pb, in_=pb_d.ap())
        nc.scalar.dma_start(out=ident, in_=id_d.ap())
        for ct in range(CT):
            nc.sync.dma_start(out=wT[:, ct, 0:C], in_=wT_r[:, ct, 0:C])
            nc.sync.dma_start(out=xT[:, ct, :], in_=xT_r[:, ct, :])
        for ct in range(CT):
            nc.sync.dma_start(out=wT[:, ct, C:2 * C],
                              in_=wT_r[:, ct, C:2 * C])
        for ct in range(CT):
            nc.sync.dma_start(out=wT[:, ct, 2 * C:3 * C],
                              in_=wT_r[:, ct, 2 * C:3 * C])
        for j in range(NT):
            nc.sync.dma_start(out=maskT[:, j, :], in_=maskT_r[:, j, :])
        for ct in range(CT):
            nc.sync.dma_start(out=pwT[:, ct, :], in_=pwT_r[:, ct, :])

        # ones column of v_ext (col D of each head block)
        nc.vector.memset(v_ext[:, :, :, D:D + 1], 1.0)

        with (
            tc.tile_pool(name="psS", bufs=2, space="PSUM") as psS,
            tc.tile_pool(name="psQV", bufs=2, space="PSUM") as psQV,
            tc.tile_pool(name="psE", bufs=1, space="PSUM") as psE,
            tc.tile_pool(name="rs", bufs=2) as rs,
        ):
            def emit_qk_tile(i, pool=None, tag="qv"):
                # qkT rows i*128..: q rows for i<6, k rows for i>=6
                if pool is not None:
                    psQ = pool.tile([128, N], F32, name="psQ", tag=tag)
                    for ct in range(CT):
                        lhsT = wT[:, ct, i * 128:(i + 1) * 128]
                        for half in range(2):
                            nc.tensor.matmul(
                                psQ[:, half * 512:(half + 1) * 512],
                                lhsT,
                                xT[:, ct, half * 512:(half + 1) * 512],
                                start=(ct == 0), stop=(ct == CT - 1),
                            )
                    nc.vector.tensor_scalar_add(
                        out=qkT[:, i, :], in0=psQ, scalar1=qb[:, i:i + 1])
                    return
                for half in range(2):
                    psQ = psQV.tile([128, 512], F32, name="psQ", tag=tag)
                    for ct in range(CT):
                        nc.tensor.matmul(
                            psQ,
                            wT[:, ct, i * 128:(i + 1) * 128],
                            xT[:, ct, half * 512:(half + 1) * 512],
                            start=(ct == 0), stop=(ct == CT - 1),
                        )
                    for piece in range(2):
                        o = half * 512 + piece * 256
                        nc.vector.tensor_scalar_add(
                            out=qkT[:, i, o:o + 256],
                            in0=psQ[:, piece * 256:(piece + 1) * 256],
                            scalar1=qb[:, i:i + 1])

            def emit_v_tile(nt):
                # two independent psum halves so production pipelines with
                # evacuation
                psVa = psQV.tile([128, 512], F32, name="psVa", tag="qv")
                for ct in range(CT):
                    nc.tensor.matmul(
                        psVa, xT[:, ct, nt * 128:(nt + 1) * 128],
                        wT[:, ct, 2 * C:2 * C + 512],
                        start=(ct == 0), stop=(ct == CT - 1),
                    )
                for piece in range(2):
                    nc.vector.tensor_copy(
                        out=v_ext[:, nt, piece * 4:piece * 4 + 4, 0:D],
                        in_=psVa[:, piece * 256:(piece + 1) * 256].rearrange(
                            "p (h d) -> p h d", d=D),
                    )
                psVb = psQV.tile([128, 512], F32, name="psVb", tag="qv")
                for ct in range(CT):
                    nc.tensor.matmul(
                        psVb[:, 0:256], xT[:, ct, nt * 128:(nt + 1) * 128],
                        wT[:, ct, 2 * C + 512:3 * C],
                        start=(ct == 0), stop=(ct == CT - 1),
                    )
                for piece in range(2):
                    nc.vector.tensor_copy(
                        out=v_ext[:, nt, 8 + piece * 2:10 + piece * 2, 0:D],
                        in_=psVb[:, piece * 128:(piece + 1) * 128].rearrange(
                            "p (h d) -> p h d", d=D),
                    )

            def emit_S(h, j):
                qt = h // 2
                po = (h % 2) * 64
                kT_ap = qkT[po:po + D, CT + qt, j * 128:(j + 1) * 128]
                psS_t = psS.tile([128, N], F32, name="psS", tag="psS")
                for half in range(2):
                    nc.tensor.matmul(
                        psS_t[:, half * 512:(half + 1) * 512],
                        kT_ap,
                        qkT[po:po + D, qt, half * 512:(half + 1) * 512],
                        start=True, stop=True,
                    )
                e_sb = work.tile([128, N], BF16, name="e_sb", tag="e_sb",
                                 bufs=8)
                nc.scalar.activation(out=e_sb, in_=psS_t, func=Exp,
                                     scale=SCALE)
                em_sb = work.tile([128, N], BF16, name="em_sb", tag="em_sb",
                                  bufs=6)
                # gpsimd is SBUF-only; it relieves DVE of mask-muls --
                # aggressively in head 0 where DVE also evacuates the v tiles
                on_pool = (j % 2 == 1) if h == 0 else False
                eng = nc.gpsimd if on_pool else nc.vector
                eng.tensor_mul(out=em_sb, in0=e_sb, in1=maskT[:, j, :])
                return em_sb

            def emit_EV(h, j, em_sb, lohi):
                # start/stop per PSUM bank (zero region), not per qc slot:
                # the first matmul's start pending-zeroes the whole bank
                for qc in range(NT):
                    buf = lohi[qc // 4]
                    nc.tensor.matmul(
                        buf[:, qc % 4, :],
                        em_sb[:, qc * 128:(qc + 1) * 128],
                        v_ext[:, j, h, :],
                        start=(j == 0 and qc % 4 == 0),
                        stop=(j == NT - 1 and qc % 4 == 3),
                    )

            def emit_recip(buf):
                rc = rs.tile([128, 4], F32, name="rc", tag="rc")
                nc.vector.reciprocal(rc, buf[:, :, D:D + 1])
                return rc

            def emit_evac4(h, half, buf, rc, on_act):
                # fused normalize+evacuate: one op for all 4 q-chunks, with
                # the per-(partition, qc) reciprocal broadcast along d via a
                # stride-0 free dim
                qc0 = half * 4
                nc.vector.tensor_tensor(
                    out=attn[:, qc0:qc0 + 4, h * D:(h + 1) * D],
                    in0=buf[:, :, 0:D],
                    in1=rc[:, :, None].broadcast_to([128, 4, D]),
                    op=mybir.AluOpType.mult,
                )

            def emit_evac_half(h, half, lohi):
                rc = emit_recip(lohi[half])
                emit_evac4(h, half, lohi[half], rc, on_act=False)

            # pair 0's qk tiles borrow the (still idle) psS slots so they
            # pipeline with the arriving weight/x loads
            # pair-0 q and k tiles interleaved per ct so both finish as
            # soon as their chunks land (each borrows one psS slot)
            psQ0 = psS.tile([128, N], F32, name="psQ0", tag="psS")
            psQ6 = psS.tile([128, N], F32, name="psQ6", tag="psS")
            for ct in range(CT):
                for i, ps in ((0, psQ0), (CT, psQ6)):
                    lhsT = wT[:, ct, i * 128:(i + 1) * 128]
                    for half in range(2):
                        nc.tensor.matmul(
                            ps[:, half * 512:(half + 1) * 512],
                            lhsT,
                            xT[:, ct, half * 512:(half + 1) * 512],
                            start=(ct == 0), stop=(ct == CT - 1),
                        )
            for piece in range(4):
                o = piece * 256
                for i, ps in ((0, psQ0), (CT, psQ6)):
                    # ACT is idle before the first exp: split the prologue
                    # evacuation across both engines to reach S(0,0) sooner
                    if piece % 2 == 0:
                        nc.scalar.activation(
                            out=qkT[:, i, o:o + 256], in_=ps[:, o:o + 256],
                            func=Ident, bias=qb[:, i:i + 1], scale=1.0)
                    else:
                        nc.vector.tensor_scalar_add(
                            out=qkT[:, i, o:o + 256], in0=ps[:, o:o + 256],
                            scalar1=qb[:, i:i + 1])

            em_q = {}  # j -> em tile awaiting EV (lag-2 pipeline)
            carry = None
            prev = None  # (h-1, its psE bufs): evacuated during head h
            for h in range(H):
                qt = h // 2
                lo = psE.tile([128, 4, D + 1], F32, name="psE_lo", tag="lo")
                hi = psE.tile([128, 4, D + 1], F32, name="psE_hi", tag="hi")
                lohi = (lo, hi)
                for j in range(NT):
                    if j == 0 and carry is not None:
                        em_q[0] = carry
                    else:
                        em_q[j] = emit_S(h, j)
                    if h == 0 and j == 5:
                        # tile 1 late in head 0: less psQV-ring contention
                        # with the v tiles
                        emit_qk_tile(qt + 1)
                    if h == 0:
                        emit_v_tile(j)
                    if h % 2 == 0 and 0 < h < H - 2 and j == 5:
                        emit_qk_tile(qt + 1)
                    if h % 2 == 1 and h + 2 < H and j == 5:
                        emit_qk_tile(CT + qt + 1)
                    # previous head's evacuation interleaves between the em
                    # muls so it doesn't monopolize DVE at the boundary
                    if prev is not None and j in (0, 1):
                        emit_evac_half(prev[0], j, prev[1])
                    if j >= 3:
                        emit_EV(h, j - 3, em_q.pop(j - 3), lohi)
                # next head's first S ahead of this head's EV tail, so the
                # exp stream never waits behind the EV/evac chain
                carry = emit_S(h + 1, 0) if h + 1 < H else None
                emit_EV(h, NT - 3, em_q.pop(NT - 3), lohi)
                emit_EV(h, NT - 2, em_q.pop(NT - 2), lohi)
                emit_EV(h, NT - 1, em_q.pop(NT - 1), lohi)
                prev = (h, lohi)
            emit_evac_half(H - 1, 0, prev[1])
            emit_evac_half(H - 1, 1, prev[1])

        # ---- transpose attn -> attnT, projection psOT[fo, n] ---------
        with (
            tc.tile_pool(name="psT", bufs=4, space="PSUM") as psT,
            tc.tile_pool(name="psO", bufs=2, space="PSUM") as psO,
        ):
            def emit_transpose(pr, qc):
                # two 128x128 transposes share one psT tile -> one evacuation
                pst = psT.tile([128, 2, 128], BF16, name="pst", tag="pst")
                for k in range(2):
                    nc.tensor.transpose(
                        pst[:, k, :],
                        attn[:, qc + k, pr * 128:(pr + 1) * 128], ident)
                if qc % 4 == 0:
                    nc.vector.tensor_copy(
                        out=attnT[:, pr, qc * 128:(qc + 2) * 128], in_=pst)
                else:
                    nc.scalar.copy(
                        out=attnT[:, pr, qc * 128:(qc + 2) * 128], in_=pst)

            # first two column-tiles of attnT ahead of the matmuls
            for pr in (0, 1):
                for qc in range(0, NT, 2):
                    emit_transpose(pr, qc)
            for fo in range(CT):
                pso = psO.tile([128, N], F32, name="pso", tag="pso")
                for fi in range(CT):
                    if fo == 0 and fi >= 2 and fi < CT:
                        # remaining transposes, pipelined two steps ahead
                        for qc in range(0, NT, 2):
                            emit_transpose(fi, qc)
                    lhsT = pwT[:, fi, fo * 128:(fo + 1) * 128]
                    for half in range(2):
                        nc.tensor.matmul(
                            pso[:, half * 512:(half + 1) * 512],
                            lhsT,
                            attnT[:, fi, half * 512:(half + 1) * 512],
                            start=(fi == 0), stop=(fi == CT - 1),
                        )
                o_sb = work.tile([128, N], F32, name="o_sb", tag="o_sb",
                                 bufs=2)
                orow = out_d.ap()[fo * 128:(fo + 1) * 128, :]
                if fo == CT - 1:
                    # half-evacuations on ACT and DVE in parallel to shorten
                    # the exposed tail chain
                    nc.scalar.activation(
                        out=o_sb[:, 0:512], in_=pso[:, 0:512], func=Ident,
                        bias=pb[:, fo:fo + 1], scale=1.0)
                    nc.vector.tensor_scalar_add(
                        out=o_sb[:, 512:1024], in0=pso[:, 512:1024],
                        scalar1=pb[:, fo:fo + 1])
                    for q in range(2):
                        sl = slice(q * 512, (q + 1) * 512)
                        nc.sync.dma_start(out=orow[:, sl], in_=o_sb[:, sl])
                else:
                    nc.scalar.activation(out=o_sb, in_=pso, func=Ident,
                                         bias=pb[:, fo:fo + 1], scale=1.0)
                    nc.sync.dma_start(out=orow, in_=o_sb)


def _host_prep_shared(qkv_w, qkv_b, proj_w, proj_b):
    wT = np.ascontiguousarray(qkv_w.T).astype(NPBF16)          # [C, 3C]
    pwT = np.ascontiguousarray(proj_w.T).astype(NPBF16)        # [C, C]
    qb_col = np.ascontiguousarray(
        qkv_b[:2 * C].reshape(12, 128).T).astype(np.float32)
    # fold the v bias through the projection: pb' = pb + proj_w @ vb
    pb_eff = proj_b + proj_w @ qkv_b[2 * C:]
    pb_col = np.ascontiguousarray(
        pb_eff.reshape(CT, 128).T).astype(np.float32)
    ident = np.eye(128, dtype=NPBF16)
    return wT, pwT, qb_col, pb_col, ident


def kernel(x, mask, qkv_w, qkv_b, proj_w, proj_b, _trace=False):
    if "nc" not in _CACHE:
        _CACHE["nc"] = _build_bass()
    nc = _CACHE["nc"]

    wT, pwT, qb_col, pb_col, ident = _host_prep_shared(
        np.asarray(qkv_w), np.asarray(qkv_b), np.asarray(proj_w),
        np.asarray(proj_b))
    x = np.asarray(x)
    mask = np.asarray(mask)

    in_maps = []
    for i in range(B):
        in_maps.append({
            "xT": np.ascontiguousarray(x[i].T).astype(NPBF16),
            "maskT": np.ascontiguousarray(mask[i].T).astype(NPBF16),
            "qkv_wT": wT,
            "proj_wT": pwT,
            "qb_col": qb_col,
            "pb_col": pb_col,
            "ident": ident,
        })
    res = run_bass_kernel_spmd(nc, in_maps, core_ids=list(range(B)),
                               trace=_trace)
    out = np.stack([res.results[i]["outT"].T for i in range(B)], axis=0)
    if _trace:
        _CACHE["last_results"] = res
    return out
